# revision 20
# baseline (speedup 1.0000x reference)
"""Trainium2 Bass kernel for nn_BoxCrossAttention_352187318473.

Math: the reference's attention has a single KV token, so the softmax over
the key axis (length 1) is exactly 1.0 and the output is independent of
x / Wp / Wq / Wk.  The whole module collapses to

    o   = ((mish(y @ W1 + b1) @ W2 + b2)[:, KV:] @ Wv + bv) @ Wo + bo
    out[b, c, w, h] = 9 * o[b, c]          (9 = kernel_size**2 positions)

Sharding: output viewed as [B*C, W*H] = [1024, 4096]; core i produces rows
[i*128, (i+1)*128) = batch i//2, channel half i%2.  Each core runs the tiny
MLP chain for its batch (activations as [128,1] columns, weights as natural
[K, M] lhsT tiles -> no transposes anywhere), then broadcasts o across the
4096 spatial positions and DMAs 2 MB out.

Weights travel as fp16 (host cast; rel err ~5e-4 vs the f32 reference,
dominated by weight rounding) packed into three [128, N] arrays so the
whole load phase is a few large DMAs.  Biases and everything after the
last matmul stay f32.
"""

import numpy as np

import concourse.bass as bass
import concourse.bacc as bacc
import concourse.tile as tile
from concourse import mybir
from concourse.bass_utils import run_bass_kernel_spmd

F32 = mybir.dt.float32
F16 = mybir.dt.float16
AF = mybir.ActivationFunctionType
ALU = mybir.AluOpType

B, C, W, H = 4, 256, 64, 64
WH = W * H            # 4096
TAU = 256
KV = 512
N_CORES = 8

# fp16 output halves the store traffic; host upcasts during unshard.
# Adds ~5e-4 absmax-relative rounding on top of the fp16-weight ~5e-4.
OUT_F16 = True
OUT_DT = None  # set below

# fp16 pack1: ycol[2] | W1 row-chunks [2*1024]
PK1_W = 2 + 2 * 1024
# fp16 pack2: W2h row-chunks [8*512]
PK2_W = 8 * 512
# fp16 pack3: Wv row-chunks [4*256] | Wo-slice row-chunks [2*128]
PK3_W = 4 * 256 + 2 * 128
# f32 bias pack: b1t[8] | b2t[4] | bvt[2] | bot[1]
PKB_W = 8 + 4 + 2 + 1

OUT_DT = F16 if OUT_F16 else F32

_nc_cache = None


def _build_nc():
    nc = bacc.Bacc(trn_type="TRN2")

    pk1 = nc.dram_tensor("pk1", [128, PK1_W], F16, kind="ExternalInput")
    pk2 = nc.dram_tensor("pk2", [128, PK2_W], F16, kind="ExternalInput")
    pk3 = nc.dram_tensor("pk3", [128, PK3_W], F16, kind="ExternalInput")
    pkb = nc.dram_tensor("pkb", [128, PKB_W], F32, kind="ExternalInput")
    outd = nc.dram_tensor("out", [128, WH], OUT_DT, kind="ExternalOutput")

    with tile.TileContext(nc) as tc:
        with (
            tc.tile_pool(name="wp", bufs=1) as wp,
            tc.tile_pool(name="ap", bufs=1) as ap,
            tc.tile_pool(name="bcp", bufs=4) as bcp,
            tc.tile_pool(name="pp", bufs=1, space="PSUM") as pp,
        ):
            p1 = wp.tile([128, PK1_W], F16, tag="p1")
            nc.sync.dma_start(out=p1, in_=pk1[:, :])
            pb = wp.tile([128, PKB_W], F32, tag="pb")
            nc.sync.dma_start(out=pb, in_=pkb[:, :])
            # W2h split into 2 group tiles so L2 trails the DMA stream
            p2g = []
            for g in range(2):
                t = wp.tile([128, 2048], F16, tag=f"p2g{g}")
                nc.sync.dma_start(out=t, in_=pk2[:, g * 2048:(g + 1) * 2048])
                p2g.append(t)
            p3 = wp.tile([128, PK3_W], F16, tag="p3")
            nc.sync.dma_start(out=p3, in_=pk3[:, :])

            y_sb = p1[:, 0:2]

            def w1(k):                      # [128,1024] chunk k, cols m*128..
                return p1[:, 2 + k * 1024: 2 + (k + 1) * 1024]

            def w2(k):                      # k-chunk k of W2h: [128, 512]
                return p2g[k // 4][:, (k % 4) * 512:(k % 4) * 512 + 512]

            def wv(k):
                return p3[:, k * 256:(k + 1) * 256]

            def wo(k):
                return p3[:, 1024 + k * 128: 1024 + (k + 1) * 128]

            b1_sb = pb[:, 0:8]
            b2_sb = pb[:, 8:12]
            bv_sb = pb[:, 12:14]
            bo_sb = pb[:, 14:15]

            # ---- L1: t1[1024] = y @ W1  (8 m-chunks, 2 k-chunks) ----
            ps_t1 = pp.tile([128, 8], F32, tag="ps_t1")
            for m in range(8):
                for k in range(2):
                    nc.tensor.matmul(
                        out=ps_t1[:, m:m + 1],
                        lhsT=w1(k)[:, m * 128:(m + 1) * 128],
                        rhs=y_sb[:, k:k + 1],
                        start=(k == 0),
                        stop=(k == 1),
                    )
            # mish(t1 + b1) = v * tanh(ln(1 + e^v)),  v = t1 + b1
            t1b = ap.tile([128, 8], F32, tag="t1b")
            nc.vector.tensor_add(out=t1b, in0=ps_t1, in1=b1_sb)
            ex = ap.tile([128, 8], F32, tag="ex")
            nc.scalar.activation(out=ex, in_=t1b, func=AF.Exp)
            sp = ap.tile([128, 8], F32, tag="sp")
            nc.scalar.activation(out=sp, in_=ex, func=AF.Ln, bias=1.0)
            th = ap.tile([128, 8], F32, tag="th")
            nc.scalar.activation(out=th, in_=sp, func=AF.Tanh)
            m1 = ap.tile([128, 8], F16, tag="m1")
            nc.vector.tensor_mul(out=m1, in0=t1b, in1=th)

            # ---- L2: kvh[512] = m1 @ W2h  (4 m-chunks, 8 k-chunks) ----
            # k-outer so each k-group's matmuls run as its W2h chunk lands;
            # one PSUM tile per m-column keeps accumulation groups disjoint.
            ps_kv = []
            for m in range(4):
                t = pp.tile([128, 1], F32, tag=f"ps_kv{m}")
                ps_kv.append(t)
            for k in range(8):
                for m in range(4):
                    nc.tensor.matmul(
                        out=ps_kv[m][:, 0:1],
                        lhsT=w2(k)[:, m * 128:(m + 1) * 128],
                        rhs=m1[:, k:k + 1],
                        start=(k == 0),
                        stop=(k == 7),
                    )
            kvt = ap.tile([128, 4], F16, tag="kvt")
            for m in range(4):
                nc.vector.tensor_add(out=kvt[:, m:m + 1], in0=ps_kv[m],
                                     in1=b2_sb[:, m:m + 1])

            # ---- L3: vp[256] = kvh @ Wv  (2 m-chunks, 4 k-chunks) ----
            ps_vp = pp.tile([128, 2], F32, tag="ps_vp")
            for m in range(2):
                for k in range(4):
                    nc.tensor.matmul(
                        out=ps_vp[:, m:m + 1],
                        lhsT=wv(k)[:, m * 128:(m + 1) * 128],
                        rhs=kvt[:, k:k + 1],
                        start=(k == 0),
                        stop=(k == 3),
                    )
            vpt = ap.tile([128, 2], F16, tag="vpt")
            nc.vector.tensor_add(out=vpt, in0=ps_vp, in1=bv_sb)

            # ---- L4: o[128] = vp @ Wo_slice  (1 m-chunk, 2 k-chunks) ----
            ps_o = pp.tile([128, 1], F32, tag="ps_o")
            for k in range(2):
                nc.tensor.matmul(
                    out=ps_o[:, 0:1],
                    lhsT=wo(k)[:, :],
                    rhs=vpt[:, k:k + 1],
                    start=(k == 0),
                    stop=(k == 1),
                )
            # o9 = (o + bo) * 9
            o9 = ap.tile([128, 1], F32, tag="o9")
            nc.vector.tensor_scalar(
                out=o9, in0=ps_o, scalar1=bo_sb[:, 0:1], scalar2=9.0,
                op0=ALU.add, op1=ALU.mult,
            )

            # ---- broadcast along free dim + store ----
            # out[p, :] = o9[p] via DVE (carrier*0 + o9); ramped chunk widths
            # so the first store DMA launches early while DVE outruns HBM.
            widths = [512, 1024, 2560]
            off = 0
            for j, cw in enumerate(widths):
                bc = bcp.tile([128, cw], OUT_DT, tag=f"bc{j}")
                for seg in range(0, cw, 2048):
                    w = min(2048, cw - seg)
                    nc.vector.tensor_scalar(
                        out=bc[:, seg:seg + w], in0=p2g[0][:, 0:w],
                        scalar1=0.0, scalar2=o9[:, 0:1],
                        op0=ALU.mult, op1=ALU.add,
                    )
                nc.sync.dma_start(out=outd[:, off:off + cw], in_=bc)
                off += cw

    return nc


def _host_in_maps(y, W1, b1, W2, b2, Wv, bv, Wo, bo):
    n = N_CORES

    def colpack(mat, kchunks):
        # [K, M] -> [128, kchunks*M] fp16, chunk k in cols k*M..(k+1)*M
        K, M = mat.shape
        assert K == kchunks * 128
        return mat.reshape(kchunks, 128, M).transpose(1, 0, 2).reshape(128, -1)

    W2h = W2[:, KV:]
    pk2 = np.ascontiguousarray(colpack(W2h, 8).astype(np.float16))
    w1p = colpack(W1, 2).astype(np.float16)          # [128, 2048]
    wvp = colpack(Wv, 4).astype(np.float16)          # [128, 1024]

    pkb = np.empty((128, PKB_W), np.float32)
    pkb[:, 0:8] = b1.reshape(8, 128).T
    pkb[:, 8:12] = b2[KV:].reshape(4, 128).T
    pkb[:, 12:14] = bv.reshape(2, 128).T

    in_maps = []
    for core in range(n):
        b_i, half = core // 2, core % 2
        ch = slice(half * 128, (half + 1) * 128)
        pk1 = np.empty((128, PK1_W), np.float16)
        pk1[:, 0:2] = y[b_i].reshape(2, 128).T.astype(np.float16)
        pk1[:, 2:] = w1p
        pk3 = np.empty((128, PK3_W), np.float16)
        pk3[:, 0:1024] = wvp
        pk3[:, 1024:] = colpack(np.ascontiguousarray(Wo[:, ch]), 2).astype(np.float16)
        pkb_i = pkb.copy()
        pkb_i[:, 14:15] = bo[ch][:, None]
        in_maps.append({"pk1": pk1, "pk2": pk2, "pk3": pk3, "pkb": pkb_i})
    return in_maps


def run(inputs, trace=False, **kw):
    global _nc_cache
    if _nc_cache is None:
        _nc_cache = _build_nc()
        _nc_cache.finalize()
    nc = _nc_cache
    in_maps = _host_in_maps(
        np.asarray(inputs["y"], np.float32),
        np.asarray(inputs["W1"], np.float32), np.asarray(inputs["b1"], np.float32),
        np.asarray(inputs["W2"], np.float32), np.asarray(inputs["b2"], np.float32),
        np.asarray(inputs["Wv"], np.float32), np.asarray(inputs["bv"], np.float32),
        np.asarray(inputs["Wo"], np.float32), np.asarray(inputs["bo"], np.float32),
    )
    res = run_bass_kernel_spmd(nc, in_maps, core_ids=list(range(N_CORES)),
                               trace=trace, **kw)
    flat = np.empty((B * C, WH), np.float32)
    for core in range(N_CORES):
        flat[core * 128:(core + 1) * 128] = res.results[core]["out"].astype(np.float32)
    out = flat.reshape(B, C, W, H)
    return out, res


def kernel(**inputs):
    out, _ = run(inputs, trace=False)
    return out
